# revision 3
# baseline (speedup 1.0000x reference)
"""Trainium2 Bass kernel for the ACTP 2-layer-LSTM + MLP rollout model (v2).

Pure data parallel across 8 NeuronCores (batch 4096 -> 512/core), weights
replicated, feature-major [feat, batch] on-chip layout, two independent
half-batch (256) recurrences zippered to keep every engine fed.

v2 changes vs v1 (2.01ms):
 - K-chunk packing: the x1 feedback (48 rows + ones + state/act rows) is
   folded into the spare partitions (72..127) of the h-tail K-chunks, so
   LSTM1 is 2 matmuls per (gate, mslot) instead of 3, LSTM2 stays 4 but
   needs no separate act chunk, FC1 drops to 2.  108 matmuls/step vs 128.
   The packed rows are written by SBUF->SBUF DMA (DMA writes have no
   32-partition alignment constraint; compute writes do).
 - Gate PSUMs live in one [128, 4, 2, BH] tile per cell so the i/f/o
   sigmoids are ONE activation instruction (the scalar engine is the
   bottleneck at ~12.8us/step; this saves ~1.5us/step of its time).
 - PSUM: one 4-bank tag per half, bufs=1: L1 gates -> L2 gates -> fc1 ->
   fc2 psums sequentially reuse the same banks; every write-after-read
   edge coincides with a real data dependency.
 - fc for half Y of step t-1 is emitted at the TOP of step t so the
   x1-feedback DMA latency is covered by the other half's matmuls.

State tile layout (per half, bf16):
  P1 [128, 2, BH]: slot0 = h1[0:128];
                   slot1 = h1[128:200] @p0, x1 @p72, ones @p120, state @p121
  P2 [128, 2, BH]: slot0 = h2[0:128];
                   slot1 = h2[128:200] @p0, x1 @p72, ones @p120, act_t @p121
LSTM1 reads P1 (x rows weighted, state rows zero); LSTM2 reads P1+P2
(x rows zero, state/act rows weighted, bias b2 on P1's ones row); FC1
reads P2 (acts rows zero, fc1 bias on P2's ones row); biases b1 on P1's
ones row.  act_t for step t+1 is DMA-prefetched after step t's LSTM2.
"""

import sys
import functools

sys.path.insert(0, "/opt/trn_rl_repo")

import numpy as np
import ml_dtypes

import concourse.bass as bass
from concourse import bacc
import concourse.tile as tile
from concourse import mybir
from concourse.bass_utils import run_bass_kernel_spmd

# model dims
T = 120
B = 4096
F = 48   # tactile feature size
A = 6    # action dim
H = 200  # LSTM hidden
CTX = 10
NSTEP = T - 1             # 119 scan steps
NOUT = NSTEP - (CTX - 1)  # 110 outputs
NCORES = 8
BL = B // NCORES          # 512 per-core batch
BH = BL // 2              # independent half-batch
HA = 128                  # M/K chunk a
HB = H - HA               # 72
XOFF = 72                 # x1 partition offset inside slot1
ONESROW = 120
AUXROW = 121              # state rows (P1) / act rows (P2)
G4 = 4 * H

LAST_RESULT = None  # BassKernelResults of the most recent run (for test.py)

Tanh = mybir.ActivationFunctionType.Tanh
Sigmoid = mybir.ActivationFunctionType.Sigmoid

bf16 = mybir.dt.bfloat16
f32 = mybir.dt.float32


def _build_nc():
    nc = bacc.Bacc()

    # ---- DRAM parameters ----
    tact = nc.declare_dram_parameter("tact", [F, CTX, BL], bf16, isOutput=False)
    acts = nc.declare_dram_parameter("acts", [A, NSTEP, BL], bf16, isOutput=False)
    # initrows: row0 = ones, rows 1..6 = state (actions[0]), row 7 = zeros
    initr = nc.declare_dram_parameter("initr", [8, BL], bf16, isOutput=False)

    wshapes = {
        "w1a": [HA, G4], "w1b": [HA, G4],
        "w2p1a": [HA, G4], "w2p1b": [HA, G4],
        "w2p2a": [HA, G4], "w2p2b": [HA, G4],
        "w3a": [HA, H], "w3b": [HA, H],
        "w4a": [HA, F], "w4b": [96, F],
    }
    wd = {k: nc.declare_dram_parameter(k, s, bf16, isOutput=False)
          for k, s in wshapes.items()}
    b4 = nc.declare_dram_parameter("b4", [F, 1], f32, isOutput=False)

    out = nc.declare_dram_parameter("out", [NOUT, F, BL], bf16, isOutput=True)

    from contextlib import ExitStack

    with tile.TileContext(nc) as tc, ExitStack() as ctx:
        wpool = ctx.enter_context(tc.tile_pool(name="wpool", bufs=1))
        stp = ctx.enter_context(tc.tile_pool(name="stp", bufs=1))
        sp = ctx.enter_context(tc.tile_pool(name="sp", bufs=2))
        pp = ctx.enter_context(tc.tile_pool(name="pp", bufs=1, space="PSUM"))

        # ---- weights to SBUF ----
        W = {}
        for k, s in wshapes.items():
            W[k] = wpool.tile(s, bf16, name=k.upper())
            nc.sync.dma_start(out=W[k], in_=wd[k][:, :])
        B4 = wpool.tile([F, 1], f32, name="B4")
        nc.sync.dma_start(out=B4, in_=b4[:, :])

        TACT = stp.tile([F, CTX, BL], bf16, name="TACT")
        ACTS = stp.tile([A, NSTEP, BL], bf16, name="ACTS")
        INITR = stp.tile([8, BL], bf16, name="INITR")
        nc.sync.dma_start(out=TACT, in_=tact[:, :, :])
        nc.sync.dma_start(out=ACTS, in_=acts[:, :, :])
        nc.sync.dma_start(out=INITR, in_=initr[:, :])

        halves = []
        for hx in range(2):
            cs = slice(hx * BH, (hx + 1) * BH)
            hh = {"cs": cs, "hx": hx}
            hh["P1"] = stp.tile([HA, 2, BH], bf16, name=f"P1_{hx}")
            hh["P2"] = stp.tile([HA, 2, BH], bf16, name=f"P2_{hx}")
            hh["c1"] = stp.tile([HA, 2, BH], bf16, name=f"c1_{hx}")
            hh["c2"] = stp.tile([HA, 2, BH], bf16, name=f"c2_{hx}")
            hh["TEF"] = stp.tile([HA, 2, BH], bf16, name=f"TEF_{hx}")
            hh["stg"] = stp.tile([F, BH], bf16, name=f"stg_{hx}")
            for k in ("P1", "P2", "c1", "c2", "TEF"):
                nc.vector.memset(hh[k], 0.0)
            nc.vector.memset(hh["stg"], 0.0)
            # ones + state/act0 rows (DMA: arbitrary partition offsets OK)
            nc.gpsimd.dma_start(out=hh["P1"][ONESROW:ONESROW + 1, 1, :],
                                in_=INITR[0:1, cs])
            nc.gpsimd.dma_start(out=hh["P2"][ONESROW:ONESROW + 1, 1, :],
                                in_=INITR[0:1, cs])
            nc.gpsimd.dma_start(out=hh["P1"][AUXROW:AUXROW + A, 1, :],
                                in_=INITR[1:1 + A, cs])
            nc.gpsimd.dma_start(out=hh["P2"][AUXROW:AUXROW + A, 1, :],
                                in_=ACTS[:, 0, cs])
            halves.append(hh)

        # pre-zero both halves' PSUM slots once: junk lanes read as 0 forever
        for hx in range(2):
            pzs = pp.tile([HA, 3, 2, BH], f32, name=f"pzs_{hx}", tag=f"s{hx}")
            pzg = pp.tile([HA, 2, BH], f32, name=f"pzg_{hx}", tag=f"g{hx}")
            nc.vector.memset(pzs, 0.0)
            nc.vector.memset(pzg, 0.0)

        # gate column base in the permuted [i, f, o, g] weight layout
        def cols(gi, m):
            base = gi * H
            return slice(base, base + HA) if m == 0 else slice(base + HA, base + H)

        def msize(m):
            return HA if m == 0 else HB

        def emit_lstm_mms(hh, wks, tag):
            """wks: list of (weight_key, rhs_ap) K-chunks.  One accumulation
            group per (gate, mslot), emitted contiguously (groups in a PSUM
            zero region must not interleave).  Sigmoid gates i/f/o land in a
            3-bank tile (one merged activation), tanh gate g in its own."""
            hx = hh["hx"]
            sgp = pp.tile([HA, 3, 2, BH], f32, name=f"s{tag}", tag=f"s{hx}")
            ggp = pp.tile([HA, 2, BH], f32, name=f"gg{tag}", tag=f"g{hx}")
            last = len(wks) - 1
            for gi in range(4):
                for m in range(2):
                    ps = (sgp[0:msize(m), gi, m, :] if gi < 3
                          else ggp[0:msize(m), m, :])
                    for j, (wk, rhs) in enumerate(wks):
                        nc.tensor.matmul(ps, W[wk][:, cols(gi, m)], rhs,
                                         start=(j == 0), stop=(j == last))
            return sgp, ggp

        def emit_cell(hh, gps, c, ha_dst, hb_dst, tag):
            sgp, ggp = gps
            sg = sp.tile([HA, 3, 2, BH], bf16, name=f"sg{tag}", tag="sg")
            gt = sp.tile([HA, 2, BH], bf16, name=f"gt{tag}", tag="gt")
            nc.scalar.activation(sg, sgp, Sigmoid)
            nc.scalar.activation(gt, ggp, Tanh)
            ig = sp.tile([HA, 2, BH], bf16, name=f"ig{tag}", tag="ig")
            fm = sp.tile([HA, 2, BH], bf16, name=f"fm{tag}", tag="fm")
            nc.vector.tensor_mul(ig, sg[:, 0, :, :], gt)
            nc.vector.tensor_mul(fm, sg[:, 1, :, :], c)
            nc.vector.tensor_add(c, fm, ig)
            tch = sp.tile([HA, 2, BH], bf16, name=f"tch{tag}", tag="tch")
            nc.scalar.activation(tch, c, Tanh)
            nc.vector.tensor_mul(ha_dst, sg[:, 2, 0, :], tch[:, 0, :])
            nc.vector.tensor_mul(hb_dst, sg[0:HB, 2, 1, :], tch[0:HB, 1, :])

        def emit_l1(hh, t):
            gps = emit_lstm_mms(hh, [("w1a", hh["P1"][:, 0, :]),
                                     ("w1b", hh["P1"][:, 1, :])],
                                f"1_{t}_{hh['hx']}")
            emit_cell(hh, gps, hh["c1"], hh["P1"][:, 0, :],
                      hh["P1"][0:HB, 1, :], f"1_{t}_{hh['hx']}")

        def emit_l2(hh, t):
            gps = emit_lstm_mms(hh, [("w2p1a", hh["P1"][:, 0, :]),
                                     ("w2p1b", hh["P1"][:, 1, :]),
                                     ("w2p2a", hh["P2"][:, 0, :]),
                                     ("w2p2b", hh["P2"][:, 1, :])],
                                f"2_{t}_{hh['hx']}")
            emit_cell(hh, gps, hh["c2"], hh["P2"][:, 0, :],
                      hh["P2"][0:HB, 1, :], f"2_{t}_{hh['hx']}")
            # prefetch act_{t+1} rows via the idle SP engine (keeps the Pool
            # queue free for the latency-critical x1 feedback triggers)
            if t + 1 < NSTEP:
                nc.sync.dma_start(out=hh["P2"][AUXROW:AUXROW + A, 1, :],
                                  in_=ACTS[:, t + 1, hh["cs"]])

        def emit_fc(hh, t):
            hx = hh["hx"]
            fcp = pp.tile([HA, 2, BH], f32, name=f"fcp_{t}_{hx}", tag=f"s{hx}")
            for m in range(2):
                mc = slice(0, HA) if m == 0 else slice(HA, H)
                nc.tensor.matmul(fcp[0:msize(m), m, :], W["w3a"][:, mc],
                                 hh["P2"][:, 0, :], start=True, stop=False)
                nc.tensor.matmul(fcp[0:msize(m), m, :], W["w3b"][:, mc],
                                 hh["P2"][:, 1, :], start=False, stop=True)
            nc.scalar.activation(hh["TEF"], fcp, Tanh)
            f2p = pp.tile([F, BH], f32, name=f"f2p_{t}_{hx}", tag=f"g{hx}")
            nc.tensor.matmul(f2p, W["w4a"], hh["TEF"][:, 0, :],
                             start=True, stop=False)
            nc.tensor.matmul(f2p, W["w4b"], hh["TEF"][0:96, 1, :],
                             start=False, stop=True)
            nc.scalar.activation(hh["stg"], f2p, Tanh, bias=B4)
            nc.gpsimd.dma_start(out=out[t - (CTX - 1), :, hh["cs"]],
                                in_=hh["stg"])
            if t < NSTEP - 1:
                # feedback x1 into the packed rows of P1/P2 (t+1 consumes)
                nc.gpsimd.dma_start(out=hh["P1"][XOFF:XOFF + F, 1, :],
                                    in_=hh["stg"])
                nc.gpsimd.dma_start(out=hh["P2"][XOFF:XOFF + F, 1, :],
                                    in_=hh["stg"])

        def emit_ctx(hh, t):
            nc.sync.dma_start(out=hh["P1"][XOFF:XOFF + F, 1, :],
                              in_=TACT[:, t, hh["cs"]])
            nc.sync.dma_start(out=hh["P2"][XOFF:XOFF + F, 1, :],
                              in_=TACT[:, t, hh["cs"]])

        X, Y = halves
        for t in range(NSTEP):
            if t - 1 >= CTX - 1:
                emit_fc(Y, t - 1)
            if t < CTX:
                emit_ctx(X, t)
                emit_ctx(Y, t)
            emit_l1(X, t)
            emit_l1(Y, t)
            emit_l2(X, t)
            emit_l2(Y, t)
            if t >= CTX - 1:
                emit_fc(X, t)
        emit_fc(Y, NSTEP - 1)

    nc.finalize()
    return nc


@functools.lru_cache(maxsize=1)
def _get_nc():
    return _build_nc()


def _prep_weights(W_ih1, W_hh1, b_ih1, b_hh1, W_ih2, W_hh2, b_ih2, b_hh2,
                  fc1_w, fc1_b, fc2_w, fc2_b):
    # gate rows reordered [i, f, o, g]
    perm = np.concatenate([np.arange(0, 200), np.arange(200, 400),
                           np.arange(600, 800), np.arange(400, 600)])
    W1i = np.asarray(W_ih1, np.float32)[perm]      # [800, 48]
    W1h = np.asarray(W_hh1, np.float32)[perm]      # [800, 200]
    b1p = (np.asarray(b_ih1) + np.asarray(b_hh1)).astype(np.float32)[perm]
    W2i = np.asarray(W_ih2, np.float32)[perm]      # [800, 248]
    W2h = np.asarray(W_hh2, np.float32)[perm]      # [800, 200]
    b2p = (np.asarray(b_ih2) + np.asarray(b_hh2)).astype(np.float32)[perm]
    Wt = W2i[:, 200:248]
    W2eff = Wt[:, 0:12] + Wt[:, 12:24] + Wt[:, 24:36] + Wt[:, 36:48]  # [800,12]
    fc1_w = np.asarray(fc1_w, np.float32)
    fc1_b = np.asarray(fc1_b, np.float32)
    fc2_w = np.asarray(fc2_w, np.float32)
    fc2_b = np.asarray(fc2_b, np.float32)

    def c(x):
        return np.ascontiguousarray(x).astype(ml_dtypes.bfloat16)

    w1b = np.zeros((HA, G4), np.float32)
    w1b[0:HB] = W1h[:, HA:H].T
    w1b[XOFF:XOFF + F] = W1i.T
    w1b[ONESROW] = b1p

    w2p1b = np.zeros((HA, G4), np.float32)
    w2p1b[0:HB] = W2i[:, HA:H].T
    w2p1b[ONESROW] = b2p
    w2p1b[AUXROW:AUXROW + A] = W2eff[:, A:2 * A].T   # state part

    w2p2b = np.zeros((HA, G4), np.float32)
    w2p2b[0:HB] = W2h[:, HA:H].T
    w2p2b[AUXROW:AUXROW + A] = W2eff[:, 0:A].T       # act part

    w3b = np.zeros((HA, H), np.float32)
    w3b[0:HB] = fc1_w[:, HA:H].T
    w3b[XOFF:XOFF + F] = fc1_w[:, H:H + F].T
    w3b[ONESROW] = fc1_b

    w4b = np.zeros((96, F), np.float32)
    w4b[0:HB] = fc2_w[:, HA:H].T

    return {
        "w1a": c(W1h[:, 0:HA].T),
        "w1b": c(w1b),
        "w2p1a": c(W2i[:, 0:HA].T),
        "w2p1b": c(w2p1b),
        "w2p2a": c(W2h[:, 0:HA].T),
        "w2p2b": c(w2p2b),
        "w3a": c(fc1_w[:, 0:HA].T),
        "w3b": c(w3b),
        "w4a": c(fc2_w[:, 0:HA].T),
        "w4b": c(w4b),
        "b4": np.ascontiguousarray(fc2_b[:, None]).astype(np.float32),
    }


def kernel(tactiles, actions, W_ih1, W_hh1, b_ih1, b_hh1,
           W_ih2, W_hh2, b_ih2, b_hh2, fc1_w, fc1_b, fc2_w, fc2_b):
    global LAST_RESULT
    tactiles = np.asarray(tactiles)
    actions = np.asarray(actions)

    wmap = _prep_weights(W_ih1, W_hh1, b_ih1, b_hh1, W_ih2, W_hh2,
                         b_ih2, b_hh2, fc1_w, fc1_b, fc2_w, fc2_b)

    in_maps = []
    for i in range(NCORES):
        s = slice(i * BL, (i + 1) * BL)
        tt = np.ascontiguousarray(
            tactiles[0:CTX, s, :].transpose(2, 0, 1)).astype(ml_dtypes.bfloat16)
        acts_T = np.ascontiguousarray(
            actions[1:T, s, :].transpose(2, 0, 1)).astype(ml_dtypes.bfloat16)
        ir = np.zeros((8, BL), np.float32)
        ir[0] = 1.0
        ir[1:1 + A] = actions[0, s, :].T
        m = {"tact": tt, "acts": acts_T,
             "initr": ir.astype(ml_dtypes.bfloat16)}
        m.update(wmap)
        in_maps.append(m)

    nc = _get_nc()
    res = run_bass_kernel_spmd(nc, in_maps, core_ids=list(range(NCORES)))
    LAST_RESULT = res

    outs = [np.asarray(r["out"]).astype(np.float32) for r in res.results]
    full = np.concatenate([o.transpose(0, 2, 1) for o in outs], axis=1)
    return np.ascontiguousarray(full)
